# revision 12
# baseline (speedup 1.0000x reference)
"""BottleneckAttn Trainium2 kernel.

Full inputs -> full output. Internally: 8-way head-parallel sharding, one
(batch, head) pair per NeuronCore. Per core, a fused transposed-attention
kernel:

  attn^T[m, n] = sum_d k[d,m] q'[d,n] + XH^T[h'(m), n] + XW^T[w'(m), n]
  P^T = exp(attn^T)            (no row-max subtraction; logits are O(10))
  out^T[dv, n] = sum_m v[m, dv] P^T[m, n] / S[n],  S[n] = sum_m P^T[m, n]

where q' = SCALE*q is folded into the q projection weights and the relative
tables are pre-scaled by 1/SCALE on host (both exact powers of two), and
  XH^T[j, n] = sum_d q[d,n] * height_rel[63 + j - n//64, d]
  XW^T[j, n] = sum_d q[d,n] * width_rel [63 + j - n%64,  d]

The bias terms are injected into the logits via PE matmuls against constant
0/1 selector matrices (E_h stacked under k in the stationary operand; E_w as
a second accumulating matmul), so the only elementwise pass over the full
16.8M-logit matrix is the single ACT exp. The softmax denominator comes for
free as a 65th row of the AV matmul (ones column appended to v^T).
"""

import sys

if "/opt/trn_rl_repo" not in sys.path:
    sys.path.insert(0, "/opt/trn_rl_repo")

import numpy as np

import concourse.bass as bass
import concourse.tile as tile
from concourse import bacc, mybir
from concourse.bass_utils import run_bass_kernel_spmd

B, C, H, W = 2, 256, 64, 64
NH, D = 4, 64
HW = H * W  # 4096
NB = 8      # n blocks of 512
NMC = 32    # m chunks of 128
FP32 = mybir.dt.float32
AF = mybir.ActivationFunctionType
FP32R = mybir.dt.float32r


def _r(ap):
    return ap.bitcast(FP32R)

_prog = None


def _build():
    nc = bacc.Bacc("TRN2", target_bir_lowering=False, debug=False)

    x_d = nc.dram_tensor("x", [2, 128, HW], FP32R, kind="ExternalInput").ap()
    wq_d = nc.dram_tensor("wq", [2, 128, D], FP32R, kind="ExternalInput").ap()
    wk_d = nc.dram_tensor("wk", [2, 128, D], FP32R, kind="ExternalInput").ap()
    wv_d = nc.dram_tensor("wv", [2, 128, D], FP32R, kind="ExternalInput").ap()
    hrel_d = nc.dram_tensor("hrel", [64, 127], FP32, kind="ExternalInput").ap()
    wrel_d = nc.dram_tensor("wrel", [64, 127], FP32, kind="ExternalInput").ap()
    eh_d = nc.dram_tensor("eh", [64, HW], FP32R, kind="ExternalInput").ap()
    ew_d = nc.dram_tensor("ew", [64, 128], FP32R, kind="ExternalInput").ap()
    ones1_d = nc.dram_tensor("ones1", [1, 64], FP32R, kind="ExternalInput").ap()
    onesv_d = nc.dram_tensor("onesv", [128, NMC], FP32R, kind="ExternalInput").ap()
    out_d = nc.dram_tensor("out", [D, HW], FP32, kind="ExternalOutput").ap()

    with tile.TileContext(nc) as tc:
        with (
            tc.tile_pool(name="const", bufs=1) as constp,
            tc.tile_pool(name="big", bufs=1) as bigp,
            tc.tile_pool(name="ptp", bufs=4) as ptp,
            tc.tile_pool(name="outp", bufs=2) as outp,
            tc.tile_pool(name="pt_psum", bufs=3, space="PSUM") as pt_psum,
            tc.tile_pool(name="av_psum", bufs=2, space="PSUM") as av_psum,
            tc.tile_pool(name="aux_psum", bufs=3, space="PSUM") as aux_psum,
        ):
            # ---------------- input tiles ----------------
            x_sb = bigp.tile([128, 2, HW], FP32R)       # x[b]: (c, n), c = t*128+p
            wq_sb = constp.tile([128, 2, D], FP32R)
            wk_sb = constp.tile([128, 2, D], FP32R)
            wv_sb = constp.tile([128, 2, D], FP32R)
            hrel_sb = constp.tile([64, 127], FP32)     # height_rel^T / SCALE
            wrel_sb = constp.tile([64, 127], FP32)
            ew_sb = constp.tile([64, 128], FP32R)       # [I64 | I64]
            # lhsT of logits mm1: rows 0:64 = k, rows 64:128 = E_h
            keh = bigp.tile([128, HW], FP32R)
            # rhs of logits mm1: rows 0:64 = q' (scaled q), rows 64:128 = XH^T
            rhs1 = bigp.tile([128, HW], FP32R)
            q_sb = bigp.tile([64, 64, 64], FP32)       # q', free = (g, w), n=g*64+w
            xw_t = bigp.tile([64, 64, 64], FP32R)       # XW^T, free = (g, w)
            v_t = bigp.tile([128, NMC, D + 1], FP32R)   # v^T chunks + ones column
            unnorm = bigp.tile([65, HW], FP32)         # rows 0:64 out^T, row 64 = S
            # (row 64 of 65-partition tiles => same ACT lane as unnorm's S row)
            onesc = constp.tile([65, 64], FP32R)
            logs = constp.tile([65, HW], FP32)
            recip = constp.tile([65, HW], FP32R)

            for t in range(2):
                nc.sync.dma_start(out=wq_sb[:, t, :], in_=wq_d[t])
                nc.sync.dma_start(out=wk_sb[:, t, :], in_=wk_d[t])
                nc.sync.dma_start(out=wv_sb[:, t, :], in_=wv_d[t])
            for cb in range(NB):
                xsl = slice(cb * 512, (cb + 1) * 512)
                for t in range(2):
                    nc.sync.dma_start(out=x_sb[:, t, xsl], in_=x_d[t, :, xsl])
            nc.sync.dma_start(out=hrel_sb[:, :], in_=hrel_d[:, :])
            nc.sync.dma_start(out=wrel_sb[:, :], in_=wrel_d[:, :])
            nc.sync.dma_start(out=keh[64:128, :], in_=eh_d[:, :])
            nc.sync.dma_start(out=ew_sb[:, :], in_=ew_d[:, :])
            nc.sync.dma_start(out=onesc[64:65, :], in_=ones1_d[:, :])
            nc.sync.dma_start(out=v_t[:, :, D], in_=onesv_d[:, :])

            # ---------------- phase 1: q', k projections ----------------
            for nb in range(NB):
                sl = slice(nb * 512, (nb + 1) * 512)
                psq = aux_psum.tile([64, 512], FP32, name="psq", tag="aux")
                for t in range(2):
                    nc.tensor.matmul(
                        psq[:, :], (wq_sb[:, t, :]), (x_sb[:, t, sl]),
                        start=(t == 0), stop=(t == 1),
                    )
                nc.vector.tensor_copy(rhs1[0:64, sl], psq[:, :])
                nc.scalar.copy(q_sb[:, 8 * nb:8 * (nb + 1), :], psq[:, :])
                psk = aux_psum.tile([64, 512], FP32, name="psk", tag="aux")
                for t in range(2):
                    nc.tensor.matmul(
                        psk[:, :], (wk_sb[:, t, :]), (x_sb[:, t, sl]),
                        start=(t == 0), stop=(t == 1),
                    )
                nc.vector.tensor_copy(keh[0:64, sl], psk[:, :])

            # ---------------- phase 2: v^T ----------------
            for mc in range(NMC):
                psv = aux_psum.tile([128, D], FP32, name="psv", tag="aux")
                for t in range(2):
                    nc.tensor.matmul(
                        psv[:, :], x_sb[:, t, mc * 128:(mc + 1) * 128],
                        wv_sb[:, t, :],
                        start=(t == 0), stop=(t == 1),
                    )
                nc.vector.tensor_copy(v_t[:, mc, 0:D], psv[:, :])

            # ---------------- phase 3: XH^T -> rhs1[64:128] ----------------
            for hh in range(64):
                psh = aux_psum.tile([128, 64], FP32, name="psh", tag="aux")
                nc.tensor.matmul(
                    psh[64:128, :], hrel_sb[:, 63 - hh:127 - hh],
                    q_sb[:, hh, :], start=True, stop=True,
                )
                nc.vector.tensor_copy(
                    rhs1[64:128, 64 * hh:64 * (hh + 1)], psh[64:128, :]
                )

            # ---------------- phase 4: XW^T ----------------
            for ww in range(64):
                psw = aux_psum.tile([64, 64], FP32, name="psw", tag="aux")
                nc.tensor.matmul(
                    psw[:, :], wrel_sb[:, 63 - ww:127 - ww],
                    q_sb[:, :, ww], start=True, stop=True,
                )
                nc.vector.tensor_copy(xw_t[:, :, ww], psw[:, :])

            # ---------------- phase 5: attention main loop ----------------
            # software pipeline: mm3 for iter i issues after mm1/mm2 of
            # iter i+2, so PE never stalls on the ACT exp of iter i.
            SKEW = 2
            for nb in range(NB):
                sl = slice(nb * 512, (nb + 1) * 512)
                av = av_psum.tile([65, 512], FP32, name="av")
                pts = {}
                for mc in range(NMC + SKEW):
                    if mc < NMC:
                        pt_ps = pt_psum.tile([128, 512], FP32, name="pt_ps")
                        nc.tensor.matmul(
                            pt_ps[:, :], keh[:, mc * 128:(mc + 1) * 128],
                            rhs1[:, sl], start=True, stop=False,
                        )
                        nc.tensor.matmul(
                            pt_ps[:, :], ew_sb[:, :],
                            xw_t[:, 8 * nb:8 * (nb + 1), :],
                            start=False, stop=True,
                        )
                        pt = ptp.tile([128, 512], FP32R, name="pt")
                        nc.scalar.activation(pt[:, :], pt_ps[:, :], AF.Exp)
                        pts[mc] = pt
                    if mc >= SKEW:
                        j = mc - SKEW
                        nc.tensor.matmul(
                            av[:, :], v_t[:, j, :], pts.pop(j)[:, :],
                            start=(j == 0), stop=(j == NMC - 1),
                        )
                nc.vector.tensor_copy(unnorm[:, sl], av[:, :])

            # ---------------- phase 6: normalize + store ----------------
            nc.scalar.activation(logs[64:65, :], unnorm[64:65, :], AF.Ln)
            nc.scalar.activation(recip[64:65, :], logs[64:65, :], AF.Exp,
                                 scale=-1.0)
            for nb in range(NB):
                sl = slice(nb * 512, (nb + 1) * 512)
                rep = aux_psum.tile([64, 512], FP32, name="rep", tag="aux")
                nc.tensor.matmul(
                    rep[:, :], onesc[64:65, :], recip[64:65, sl],
                    start=True, stop=True
                )
                ot = outp.tile([64, 512], FP32, name="ot")
                nc.vector.tensor_mul(ot[:, :], unnorm[0:64, sl], rep[:, :])
                nc.sync.dma_start(out=out_d[:, sl], in_=ot[:, :])

    nc.finalize()
    return nc


def _get_program():
    global _prog
    if _prog is None:
        _prog = _build()
    return _prog


def _make_in_maps(x, qkv_w, height_rel, width_rel):
    x = np.ascontiguousarray(np.asarray(x, dtype=np.float32))
    qkv_w = np.ascontiguousarray(np.asarray(qkv_w, dtype=np.float32))
    height_rel = np.asarray(height_rel, dtype=np.float32)
    width_rel = np.asarray(width_rel, dtype=np.float32)

    # exact power-of-two rescale: q' = q/8 folded into wq, rel tables * 8
    hrel_t = np.ascontiguousarray((height_rel * np.float32(8.0)).T)  # (64, 127)
    wrel_t = np.ascontiguousarray((width_rel * np.float32(8.0)).T)

    eh = np.zeros((64, HW), dtype=np.float32)
    for j in range(64):
        eh[j, j * 64:(j + 1) * 64] = 1.0
    ew = np.zeros((64, 128), dtype=np.float32)
    idx = np.arange(64)
    ew[idx, idx] = 1.0
    ew[idx, 64 + idx] = 1.0

    in_maps = []
    for core in range(8):
        b, h = divmod(core, 4)
        wq = qkv_w[D * h:D * (h + 1)] * np.float32(0.125)       # (64, 256)
        wk = qkv_w[C + D * h:C + D * (h + 1)]
        wv = qkv_w[2 * C + D * h:2 * C + D * (h + 1)]
        in_maps.append({
            "x": np.ascontiguousarray(x[b].reshape(2, 128, HW)),
            "wq": np.ascontiguousarray(wq.T.reshape(2, 128, D)),
            "wk": np.ascontiguousarray(wk.T.reshape(2, 128, D)),
            "wv": np.ascontiguousarray(wv.T.reshape(2, 128, D)),
            "hrel": hrel_t,
            "wrel": wrel_t,
            "eh": eh,
            "ew": ew,
            "ones1": np.ones((1, 64), dtype=np.float32),
            "onesv": np.ones((128, NMC), dtype=np.float32),
        })
    return in_maps


def _assemble(results):
    out = np.empty((B, C, H, W), dtype=np.float32)
    for core in range(8):
        b, h = divmod(core, 4)
        out[b, D * h:D * (h + 1)] = np.asarray(
            results[core]["out"], dtype=np.float32
        ).reshape(D, H, W)
    return out


def kernel(x, qkv_w, height_rel, width_rel):
    nc = _get_program()
    in_maps = _make_in_maps(x, qkv_w, height_rel, width_rel)
    res = run_bass_kernel_spmd(nc, in_maps, list(range(8)))
    return _assemble(res.results)


if __name__ == "__main__":
    rng = np.random.default_rng(0)
    xs = rng.standard_normal((B, C, H, W), dtype=np.float32)
    ws = rng.standard_normal((768, C), dtype=np.float32) * C ** -0.5
    hr = rng.standard_normal((2 * H - 1, D), dtype=np.float32) * D ** -0.5
    wr = rng.standard_normal((2 * W - 1, D), dtype=np.float32) * D ** -0.5
    o = kernel(xs, ws, hr, wr)
    print(o.shape, o.dtype, float(np.abs(o).mean()))


# revision 13
# speedup vs baseline: 4.6076x; 4.6076x over previous
"""BottleneckAttn Trainium2 kernel.

Full inputs -> full output. Internally: 8-way head-parallel sharding, one
(batch, head) pair per NeuronCore. Per core, a fused transposed-attention
kernel:

  attn^T[m, n] = sum_d k[d,m] q'[d,n] + XH^T[h'(m), n] + XW^T[w'(m), n]
  P^T = exp(attn^T)            (no row-max subtraction; logits are O(10))
  out^T[dv, n] = sum_m v[m, dv] P^T[m, n] / S[n],  S[n] = sum_m P^T[m, n]

where q' = SCALE*q is folded into the q projection weights and the relative
tables are pre-scaled by 1/SCALE on host (both exact powers of two), and
  XH^T[j, n] = sum_d q[d,n] * height_rel[63 + j - n//64, d]
  XW^T[j, n] = sum_d q[d,n] * width_rel [63 + j - n%64,  d]

The bias terms are injected into the logits via PE matmuls against constant
0/1 selector matrices (E_h stacked under k in the stationary operand; E_w as
a second accumulating matmul), so the only elementwise pass over the full
16.8M-logit matrix is the single ACT exp. The softmax denominator comes for
free as a 65th row of the AV matmul (ones column appended to v^T).
"""

import sys

if "/opt/trn_rl_repo" not in sys.path:
    sys.path.insert(0, "/opt/trn_rl_repo")

import numpy as np

import concourse.bass as bass
import concourse.tile as tile
from concourse import bacc, mybir
from concourse.bass_utils import run_bass_kernel_spmd

B, C, H, W = 2, 256, 64, 64
NH, D = 4, 64
HW = H * W  # 4096
NB = 8      # n blocks of 512
NMC = 32    # m chunks of 128
FP32 = mybir.dt.float32
AF = mybir.ActivationFunctionType
FP32R = mybir.dt.float32r


def _r(ap):
    return ap.bitcast(FP32R)

_prog = None


def _build(loop_k=1):
    nc = bacc.Bacc("TRN2", target_bir_lowering=False, debug=False)

    x_d = nc.dram_tensor("x", [2, 128, HW], FP32R, kind="ExternalInput").ap()
    wq_d = nc.dram_tensor("wq", [2, 128, D], FP32R, kind="ExternalInput").ap()
    wk_d = nc.dram_tensor("wk", [2, 128, D], FP32R, kind="ExternalInput").ap()
    wv_d = nc.dram_tensor("wv", [2, 128, D], FP32R, kind="ExternalInput").ap()
    hrel_d = nc.dram_tensor("hrel", [64, 127], FP32, kind="ExternalInput").ap()
    wrel_d = nc.dram_tensor("wrel", [64, 127], FP32, kind="ExternalInput").ap()
    eh_d = nc.dram_tensor("eh", [64, HW], FP32R, kind="ExternalInput").ap()
    ew_d = nc.dram_tensor("ew", [64, 128], FP32R, kind="ExternalInput").ap()
    ones1_d = nc.dram_tensor("ones1", [1, 64], FP32R, kind="ExternalInput").ap()
    onesv_d = nc.dram_tensor("onesv", [128, NMC], FP32R, kind="ExternalInput").ap()
    out_d = nc.dram_tensor("out", [D, HW], FP32, kind="ExternalOutput").ap()

    with tile.TileContext(nc) as tc:
        import contextlib
        loop_ctx = tc.For_i(0, loop_k, 1) if loop_k > 1 else (
            contextlib.nullcontext())
        with (
            tc.tile_pool(name="const", bufs=1) as constp,
            tc.tile_pool(name="big", bufs=1) as bigp,
            tc.tile_pool(name="ptp", bufs=4) as ptp,
            tc.tile_pool(name="outp", bufs=2) as outp,
            tc.tile_pool(name="pt_psum", bufs=3, space="PSUM") as pt_psum,
            tc.tile_pool(name="av_psum", bufs=2, space="PSUM") as av_psum,
            tc.tile_pool(name="aux_psum", bufs=3, space="PSUM") as aux_psum,
            loop_ctx,
        ):
            # ---------------- input tiles ----------------
            x_sb = bigp.tile([128, 2, HW], FP32R)       # x[b]: (c, n), c = t*128+p
            wq_sb = constp.tile([128, 2, D], FP32R)
            wk_sb = constp.tile([128, 2, D], FP32R)
            wv_sb = constp.tile([128, 2, D], FP32R)
            hrel_sb = constp.tile([64, 127], FP32)     # height_rel^T / SCALE
            wrel_sb = constp.tile([64, 127], FP32)
            ew_sb = constp.tile([64, 128], FP32R)       # [I64 | I64]
            # lhsT of logits mm1: rows 0:64 = k, rows 64:128 = E_h
            keh = bigp.tile([128, HW], FP32R)
            # rhs of logits mm1: rows 0:64 = q' (scaled q), rows 64:128 = XH^T
            rhs1 = bigp.tile([128, HW], FP32R)
            q_sb = bigp.tile([64, 64, 64], FP32)       # q', free = (g, w), n=g*64+w
            xw_t = bigp.tile([64, 64, 64], FP32R)       # XW^T, free = (g, w)
            v_t = bigp.tile([128, NMC, D + 1], FP32R)   # v^T chunks + ones column
            unnorm = bigp.tile([65, HW], FP32)         # rows 0:64 out^T, row 64 = S
            # (row 64 of 65-partition tiles => same ACT lane as unnorm's S row)
            onesc = constp.tile([65, 64], FP32R)
            logs = constp.tile([65, HW], FP32)
            recip = constp.tile([65, HW], FP32R)

            for t in range(2):
                nc.sync.dma_start(out=wq_sb[:, t, :], in_=wq_d[t])
                nc.sync.dma_start(out=wk_sb[:, t, :], in_=wk_d[t])
                nc.sync.dma_start(out=wv_sb[:, t, :], in_=wv_d[t])
            for cb in range(NB):
                xsl = slice(cb * 512, (cb + 1) * 512)
                for t in range(2):
                    nc.sync.dma_start(out=x_sb[:, t, xsl], in_=x_d[t, :, xsl])
            nc.sync.dma_start(out=hrel_sb[:, :], in_=hrel_d[:, :])
            nc.sync.dma_start(out=wrel_sb[:, :], in_=wrel_d[:, :])
            nc.sync.dma_start(out=keh[64:128, :], in_=eh_d[:, :])
            nc.sync.dma_start(out=ew_sb[:, :], in_=ew_d[:, :])
            nc.sync.dma_start(out=onesc[64:65, :], in_=ones1_d[:, :])
            nc.sync.dma_start(out=v_t[:, :, D], in_=onesv_d[:, :])

            # ---------------- phase 1: q', k projections ----------------
            for nb in range(NB):
                sl = slice(nb * 512, (nb + 1) * 512)
                psq = aux_psum.tile([64, 512], FP32, name="psq", tag="aux")
                for t in range(2):
                    nc.tensor.matmul(
                        psq[:, :], (wq_sb[:, t, :]), (x_sb[:, t, sl]),
                        start=(t == 0), stop=(t == 1),
                    )
                nc.vector.tensor_copy(rhs1[0:64, sl], psq[:, :])
                nc.scalar.copy(q_sb[:, 8 * nb:8 * (nb + 1), :], psq[:, :])
                psk = aux_psum.tile([64, 512], FP32, name="psk", tag="aux")
                for t in range(2):
                    nc.tensor.matmul(
                        psk[:, :], (wk_sb[:, t, :]), (x_sb[:, t, sl]),
                        start=(t == 0), stop=(t == 1),
                    )
                nc.vector.tensor_copy(keh[0:64, sl], psk[:, :])

            # ---------------- phase 2: v^T ----------------
            for mc in range(NMC):
                psv = aux_psum.tile([128, D], FP32, name="psv", tag="aux")
                for t in range(2):
                    nc.tensor.matmul(
                        psv[:, :], x_sb[:, t, mc * 128:(mc + 1) * 128],
                        wv_sb[:, t, :],
                        start=(t == 0), stop=(t == 1),
                    )
                nc.vector.tensor_copy(v_t[:, mc, 0:D], psv[:, :])

            # ---------------- phase 3: XH^T -> rhs1[64:128] ----------------
            for hh in range(64):
                psh = aux_psum.tile([128, 64], FP32, name="psh", tag="aux")
                nc.tensor.matmul(
                    psh[64:128, :], hrel_sb[:, 63 - hh:127 - hh],
                    q_sb[:, hh, :], start=True, stop=True,
                )
                nc.vector.tensor_copy(
                    rhs1[64:128, 64 * hh:64 * (hh + 1)], psh[64:128, :]
                )

            # ---------------- phase 4: XW^T ----------------
            for ww in range(64):
                psw = aux_psum.tile([64, 64], FP32, name="psw", tag="aux")
                nc.tensor.matmul(
                    psw[:, :], wrel_sb[:, 63 - ww:127 - ww],
                    q_sb[:, :, ww], start=True, stop=True,
                )
                nc.vector.tensor_copy(xw_t[:, :, ww], psw[:, :])

            # ---------------- phase 5: attention main loop ----------------
            # software pipeline: mm3 for iter i issues after mm1/mm2 of
            # iter i+2, so PE never stalls on the ACT exp of iter i.
            SKEW = 2
            for nb in range(NB):
                sl = slice(nb * 512, (nb + 1) * 512)
                av = av_psum.tile([65, 512], FP32, name="av")
                pts = {}
                for mc in range(NMC + SKEW):
                    if mc < NMC:
                        pt_ps = pt_psum.tile([128, 512], FP32, name="pt_ps")
                        nc.tensor.matmul(
                            pt_ps[:, :], keh[:, mc * 128:(mc + 1) * 128],
                            rhs1[:, sl], start=True, stop=False,
                        )
                        nc.tensor.matmul(
                            pt_ps[:, :], ew_sb[:, :],
                            xw_t[:, 8 * nb:8 * (nb + 1), :],
                            start=False, stop=True,
                        )
                        pt = ptp.tile([128, 512], FP32R, name="pt")
                        nc.scalar.activation(pt[:, :], pt_ps[:, :], AF.Exp)
                        pts[mc] = pt
                    if mc >= SKEW:
                        j = mc - SKEW
                        nc.tensor.matmul(
                            av[:, :], v_t[:, j, :], pts.pop(j)[:, :],
                            start=(j == 0), stop=(j == NMC - 1),
                        )
                nc.vector.tensor_copy(unnorm[:, sl], av[:, :])

            # ---------------- phase 6: normalize + store ----------------
            nc.scalar.activation(logs[64:65, :], unnorm[64:65, :], AF.Ln)
            nc.scalar.activation(recip[64:65, :], logs[64:65, :], AF.Exp,
                                 scale=-1.0)
            for nb in range(NB):
                sl = slice(nb * 512, (nb + 1) * 512)
                rep = aux_psum.tile([64, 512], FP32, name="rep", tag="aux")
                nc.tensor.matmul(
                    rep[:, :], onesc[64:65, :], recip[64:65, sl],
                    start=True, stop=True
                )
                ot = outp.tile([64, 512], FP32, name="ot")
                nc.vector.tensor_mul(ot[:, :], unnorm[0:64, sl], rep[:, :])
                nc.sync.dma_start(out=out_d[:, sl], in_=ot[:, :])

    nc.finalize()
    return nc


def _get_program():
    global _prog
    if _prog is None:
        _prog = _build()
    return _prog


def _make_in_maps(x, qkv_w, height_rel, width_rel):
    x = np.ascontiguousarray(np.asarray(x, dtype=np.float32))
    qkv_w = np.ascontiguousarray(np.asarray(qkv_w, dtype=np.float32))
    height_rel = np.asarray(height_rel, dtype=np.float32)
    width_rel = np.asarray(width_rel, dtype=np.float32)

    # exact power-of-two rescale: q' = q/8 folded into wq, rel tables * 8
    hrel_t = np.ascontiguousarray((height_rel * np.float32(8.0)).T)  # (64, 127)
    wrel_t = np.ascontiguousarray((width_rel * np.float32(8.0)).T)

    eh = np.zeros((64, HW), dtype=np.float32)
    for j in range(64):
        eh[j, j * 64:(j + 1) * 64] = 1.0
    ew = np.zeros((64, 128), dtype=np.float32)
    idx = np.arange(64)
    ew[idx, idx] = 1.0
    ew[idx, 64 + idx] = 1.0

    in_maps = []
    for core in range(8):
        b, h = divmod(core, 4)
        wq = qkv_w[D * h:D * (h + 1)] * np.float32(0.125)       # (64, 256)
        wk = qkv_w[C + D * h:C + D * (h + 1)]
        wv = qkv_w[2 * C + D * h:2 * C + D * (h + 1)]
        in_maps.append({
            "x": np.ascontiguousarray(x[b].reshape(2, 128, HW)),
            "wq": np.ascontiguousarray(wq.T.reshape(2, 128, D)),
            "wk": np.ascontiguousarray(wk.T.reshape(2, 128, D)),
            "wv": np.ascontiguousarray(wv.T.reshape(2, 128, D)),
            "hrel": hrel_t,
            "wrel": wrel_t,
            "eh": eh,
            "ew": ew,
            "ones1": np.ones((1, 64), dtype=np.float32),
            "onesv": np.ones((128, NMC), dtype=np.float32),
        })
    return in_maps


def _assemble(results):
    out = np.empty((B, C, H, W), dtype=np.float32)
    for core in range(8):
        b, h = divmod(core, 4)
        out[b, D * h:D * (h + 1)] = np.asarray(
            results[core]["out"], dtype=np.float32
        ).reshape(D, H, W)
    return out


def kernel(x, qkv_w, height_rel, width_rel):
    nc = _get_program()
    in_maps = _make_in_maps(x, qkv_w, height_rel, width_rel)
    res = run_bass_kernel_spmd(nc, in_maps, list(range(8)))
    return _assemble(res.results)


if __name__ == "__main__":
    rng = np.random.default_rng(0)
    xs = rng.standard_normal((B, C, H, W), dtype=np.float32)
    ws = rng.standard_normal((768, C), dtype=np.float32) * C ** -0.5
    hr = rng.standard_normal((2 * H - 1, D), dtype=np.float32) * D ** -0.5
    wr = rng.standard_normal((2 * W - 1, D), dtype=np.float32) * D ** -0.5
    o = kernel(xs, ws, hr, wr)
    print(o.shape, o.dtype, float(np.abs(o).mean()))


# revision 15
# speedup vs baseline: 8.5422x; 1.8539x over previous
"""BottleneckAttn Trainium2 kernel.

Full inputs -> full output. Internally: 8-way head-parallel sharding, one
(batch, head) pair per NeuronCore. Per core, a fused transposed-attention
kernel:

  attn^T[m, n] = sum_d k[d,m] q'[d,n] + XH^T[h'(m), n] + XW^T[w'(m), n]
  P^T = exp(attn^T)            (no row-max subtraction; logits are O(10))
  out^T[dv, n] = sum_m v[m, dv] P^T[m, n] / S[n],  S[n] = sum_m P^T[m, n]

where q' = SCALE*q is folded into the q projection weights and the relative
tables are pre-scaled by 1/SCALE on host (both exact powers of two), and
  XH^T[j, n] = sum_d q[d,n] * height_rel[63 + j - n//64, d]
  XW^T[j, n] = sum_d q[d,n] * width_rel [63 + j - n%64,  d]

The bias terms are injected into the logits via PE matmuls against constant
0/1 selector matrices (E_h stacked under k in the stationary operand; E_w as
a second accumulating matmul), so the only elementwise pass over the full
16.8M-logit matrix is the single ACT exp. The softmax denominator comes for
free as a 65th row of the AV matmul (ones column appended to v^T).
"""

import sys

if "/opt/trn_rl_repo" not in sys.path:
    sys.path.insert(0, "/opt/trn_rl_repo")

import numpy as np

import concourse.bass as bass
import concourse.tile as tile
from concourse import bacc, mybir
from concourse.bass_utils import run_bass_kernel_spmd

B, C, H, W = 2, 256, 64, 64
NH, D = 4, 64
HW = H * W  # 4096
NB = 8      # n blocks of 512
NMC = 32    # m chunks of 128
FP32 = mybir.dt.float32
AF = mybir.ActivationFunctionType
FP32R = mybir.dt.float32r


def _r(ap):
    return ap.bitcast(FP32R)

_prog = None


def _build(loop_k=1):
    nc = bacc.Bacc("TRN2", target_bir_lowering=False, debug=False)

    x_d = nc.dram_tensor("x", [2, 128, HW], FP32R, kind="ExternalInput").ap()
    wq_d = nc.dram_tensor("wq", [2, 128, D], FP32R, kind="ExternalInput").ap()
    wk_d = nc.dram_tensor("wk", [2, 128, D], FP32R, kind="ExternalInput").ap()
    wv_d = nc.dram_tensor("wv", [2, 128, D], FP32R, kind="ExternalInput").ap()
    hrel_d = nc.dram_tensor("hrel", [64, 127], FP32, kind="ExternalInput").ap()
    wrel_d = nc.dram_tensor("wrel", [64, 127], FP32, kind="ExternalInput").ap()
    eh_d = nc.dram_tensor("eh", [64, HW], FP32R, kind="ExternalInput").ap()
    ew_d = nc.dram_tensor("ew", [64, 128], FP32R, kind="ExternalInput").ap()
    ones1_d = nc.dram_tensor("ones1", [1, 64], FP32R, kind="ExternalInput").ap()
    onesv_d = nc.dram_tensor("onesv", [128, NMC], FP32R, kind="ExternalInput").ap()
    out_d = nc.dram_tensor("out", [D, HW], FP32, kind="ExternalOutput").ap()

    with tile.TileContext(nc) as tc:
        import contextlib
        loop_ctx = tc.For_i(0, loop_k, 1) if loop_k > 1 else (
            contextlib.nullcontext())
        with (
            tc.tile_pool(name="const", bufs=1) as constp,
            tc.tile_pool(name="big", bufs=1) as bigp,
            tc.tile_pool(name="ptp", bufs=2) as ptp,
            tc.tile_pool(name="outp", bufs=2) as outp,
            tc.tile_pool(name="pt_psum", bufs=2, space="PSUM") as pt_psum,
            tc.tile_pool(name="aux_psum", bufs=2, space="PSUM") as aux_psum,
            loop_ctx,
        ):
            # ---------------- input tiles ----------------
            x_sb = bigp.tile([128, 2, HW], FP32R)       # x[b]: (c, n), c = t*128+p
            wq_sb = constp.tile([128, 2, D], FP32R)
            wk_sb = constp.tile([128, 2, D], FP32R)
            wv_sb = constp.tile([128, 2, D], FP32R)
            hrel_sb = constp.tile([64, 127], FP32)     # height_rel^T / SCALE
            wrel_sb = constp.tile([64, 127], FP32)
            ew_sb = constp.tile([64, 128], FP32R)       # [I64 | I64]
            # lhsT of logits mm1: rows 0:64 = k, rows 64:128 = E_h
            keh = bigp.tile([128, HW], FP32R)
            # rhs of logits mm1: rows 0:64 = q' (scaled q), rows 64:128 = XH^T
            rhs1 = bigp.tile([128, HW], FP32R)
            q_sb = bigp.tile([64, 64, 64], FP32)       # q', free = (g, w), n=g*64+w
            xw_t = bigp.tile([64, 64, 64], FP32R)       # XW^T, free = (g, w)
            v_t = bigp.tile([128, NMC, D + 1], FP32R)   # v^T chunks + ones column
            unnorm = bigp.tile([65, HW], FP32)         # rows 0:64 out^T, row 64 = S
            # (row 64 of 65-partition tiles => same ACT lane as unnorm's S row)
            onesc = constp.tile([65, 64], FP32R)
            logs = constp.tile([65, HW], FP32)
            recip = constp.tile([65, HW], FP32R)

            for t in range(2):
                nc.sync.dma_start(out=wq_sb[:, t, :], in_=wq_d[t])
                nc.sync.dma_start(out=wk_sb[:, t, :], in_=wk_d[t])
                nc.sync.dma_start(out=wv_sb[:, t, :], in_=wv_d[t])
            for cb in range(NB):
                xsl = slice(cb * 512, (cb + 1) * 512)
                for t in range(2):
                    nc.sync.dma_start(out=x_sb[:, t, xsl], in_=x_d[t, :, xsl])
            nc.sync.dma_start(out=hrel_sb[:, :], in_=hrel_d[:, :])
            nc.sync.dma_start(out=wrel_sb[:, :], in_=wrel_d[:, :])
            nc.sync.dma_start(out=keh[64:128, :], in_=eh_d[:, :])
            nc.sync.dma_start(out=ew_sb[:, :], in_=ew_d[:, :])
            nc.sync.dma_start(out=onesc[64:65, :], in_=ones1_d[:, :])
            nc.sync.dma_start(out=v_t[:, :, D], in_=onesv_d[:, :])

            # ---------------- phase 1: q', k projections ----------------
            for nb in range(NB):
                sl = slice(nb * 512, (nb + 1) * 512)
                psq = aux_psum.tile([64, 512], FP32, name="psq", tag="aux")
                for t in range(2):
                    nc.tensor.matmul(
                        psq[:, :], (wq_sb[:, t, :]), (x_sb[:, t, sl]),
                        start=(t == 0), stop=(t == 1),
                    )
                nc.vector.tensor_copy(rhs1[0:64, sl], psq[:, :])
                nc.scalar.copy(q_sb[:, 8 * nb:8 * (nb + 1), :], psq[:, :])
                psk = aux_psum.tile([64, 512], FP32, name="psk", tag="aux")
                for t in range(2):
                    nc.tensor.matmul(
                        psk[:, :], (wk_sb[:, t, :]), (x_sb[:, t, sl]),
                        start=(t == 0), stop=(t == 1),
                    )
                nc.vector.tensor_copy(keh[0:64, sl], psk[:, :])

            # ---------------- phase 2: v^T ----------------
            for mc in range(NMC):
                psv = aux_psum.tile([128, D], FP32, name="psv", tag="aux")
                for t in range(2):
                    nc.tensor.matmul(
                        psv[:, :], x_sb[:, t, mc * 128:(mc + 1) * 128],
                        wv_sb[:, t, :],
                        start=(t == 0), stop=(t == 1),
                    )
                nc.vector.tensor_copy(v_t[:, mc, 0:D], psv[:, :])

            # ---------------- phase 3: XH^T -> rhs1[64:128] ----------------
            for hh in range(64):
                psh = aux_psum.tile([128, 64], FP32, name="psh", tag="aux")
                nc.tensor.matmul(
                    psh[64:128, :], hrel_sb[:, 63 - hh:127 - hh],
                    q_sb[:, hh, :], start=True, stop=True,
                )
                nc.vector.tensor_copy(
                    rhs1[64:128, 64 * hh:64 * (hh + 1)], psh[64:128, :]
                )

            # ---------------- phase 4: XW^T ----------------
            for ww in range(64):
                psw = aux_psum.tile([64, 64], FP32, name="psw", tag="aux")
                nc.tensor.matmul(
                    psw[:, :], wrel_sb[:, 63 - ww:127 - ww],
                    q_sb[:, :, ww], start=True, stop=True,
                )
                nc.vector.tensor_copy(xw_t[:, :, ww], psw[:, :])

            # ---------------- phase 5: attention main loop ----------------
            # nb processed in pairs so each stationary-weight load (keh
            # chunk, ew, v_t chunk) serves two N=512 matmuls; mm3 for
            # chunk j issues one pair-step later so PE never waits on exp.
            SKEW = 1
            for nbp in range(NB // 2):
                nbs = (2 * nbp, 2 * nbp + 1)
                sls = [slice(nb * 512, (nb + 1) * 512) for nb in nbs]
                avs = [
                    aux_psum.tile([65, 512], FP32, name=f"av{i}", tag="aux")
                    for i in range(2)
                ]
                pts = {}
                for mc in range(NMC + SKEW):
                    if mc < NMC:
                        pps = [
                            pt_psum.tile([128, 512], FP32, name=f"pt_ps{i}")
                            for i in range(2)
                        ]
                        for i in range(2):
                            nc.tensor.matmul(
                                pps[i][:, :], keh[:, mc * 128:(mc + 1) * 128],
                                rhs1[:, sls[i]], start=True, stop=False,
                            )
                        for i in range(2):
                            nc.tensor.matmul(
                                pps[i][:, :], ew_sb[:, :],
                                xw_t[:, 8 * nbs[i]:8 * (nbs[i] + 1), :],
                                start=False, stop=True,
                            )
                        cur = []
                        for i in range(2):
                            pt = ptp.tile([128, 512], FP32R, name=f"pt{i}")
                            nc.scalar.activation(pt[:, :], pps[i][:, :], AF.Exp)
                            cur.append(pt)
                        pts[mc] = cur
                    if mc >= SKEW:
                        j = mc - SKEW
                        ptj = pts.pop(j)
                        for i in range(2):
                            nc.tensor.matmul(
                                avs[i][:, :], v_t[:, j, :], ptj[i][:, :],
                                start=(j == 0), stop=(j == NMC - 1),
                            )
                for i in range(2):
                    nc.vector.tensor_copy(unnorm[:, sls[i]], avs[i][:, :])

            # ---------------- phase 6: normalize + store ----------------
            nc.scalar.activation(logs[64:65, :], unnorm[64:65, :], AF.Ln)
            nc.scalar.activation(recip[64:65, :], logs[64:65, :], AF.Exp,
                                 scale=-1.0)
            for nb in range(NB):
                sl = slice(nb * 512, (nb + 1) * 512)
                rep = aux_psum.tile([64, 512], FP32, name="rep", tag="aux")
                nc.tensor.matmul(
                    rep[:, :], onesc[64:65, :], recip[64:65, sl],
                    start=True, stop=True
                )
                ot = outp.tile([64, 512], FP32, name="ot")
                nc.vector.tensor_mul(ot[:, :], unnorm[0:64, sl], rep[:, :])
                nc.sync.dma_start(out=out_d[:, sl], in_=ot[:, :])

    nc.finalize()
    return nc


def _get_program():
    global _prog
    if _prog is None:
        _prog = _build()
    return _prog


def _make_in_maps(x, qkv_w, height_rel, width_rel):
    x = np.ascontiguousarray(np.asarray(x, dtype=np.float32))
    qkv_w = np.ascontiguousarray(np.asarray(qkv_w, dtype=np.float32))
    height_rel = np.asarray(height_rel, dtype=np.float32)
    width_rel = np.asarray(width_rel, dtype=np.float32)

    # exact power-of-two rescale: q' = q/8 folded into wq, rel tables * 8
    hrel_t = np.ascontiguousarray((height_rel * np.float32(8.0)).T)  # (64, 127)
    wrel_t = np.ascontiguousarray((width_rel * np.float32(8.0)).T)

    eh = np.zeros((64, HW), dtype=np.float32)
    for j in range(64):
        eh[j, j * 64:(j + 1) * 64] = 1.0
    ew = np.zeros((64, 128), dtype=np.float32)
    idx = np.arange(64)
    ew[idx, idx] = 1.0
    ew[idx, 64 + idx] = 1.0

    in_maps = []
    for core in range(8):
        b, h = divmod(core, 4)
        wq = qkv_w[D * h:D * (h + 1)] * np.float32(0.125)       # (64, 256)
        wk = qkv_w[C + D * h:C + D * (h + 1)]
        wv = qkv_w[2 * C + D * h:2 * C + D * (h + 1)]
        in_maps.append({
            "x": np.ascontiguousarray(x[b].reshape(2, 128, HW)),
            "wq": np.ascontiguousarray(wq.T.reshape(2, 128, D)),
            "wk": np.ascontiguousarray(wk.T.reshape(2, 128, D)),
            "wv": np.ascontiguousarray(wv.T.reshape(2, 128, D)),
            "hrel": hrel_t,
            "wrel": wrel_t,
            "eh": eh,
            "ew": ew,
            "ones1": np.ones((1, 64), dtype=np.float32),
            "onesv": np.ones((128, NMC), dtype=np.float32),
        })
    return in_maps


def _assemble(results):
    out = np.empty((B, C, H, W), dtype=np.float32)
    for core in range(8):
        b, h = divmod(core, 4)
        out[b, D * h:D * (h + 1)] = np.asarray(
            results[core]["out"], dtype=np.float32
        ).reshape(D, H, W)
    return out


def kernel(x, qkv_w, height_rel, width_rel):
    nc = _get_program()
    in_maps = _make_in_maps(x, qkv_w, height_rel, width_rel)
    res = run_bass_kernel_spmd(nc, in_maps, list(range(8)))
    return _assemble(res.results)


if __name__ == "__main__":
    rng = np.random.default_rng(0)
    xs = rng.standard_normal((B, C, H, W), dtype=np.float32)
    ws = rng.standard_normal((768, C), dtype=np.float32) * C ** -0.5
    hr = rng.standard_normal((2 * H - 1, D), dtype=np.float32) * D ** -0.5
    wr = rng.standard_normal((2 * W - 1, D), dtype=np.float32) * D ** -0.5
    o = kernel(xs, ws, hr, wr)
    print(o.shape, o.dtype, float(np.abs(o).mean()))
